# revision 1
# baseline (speedup 1.0000x reference)
"""Causal self-attention (B=4, T=2048, C=2048, H=16, HD=128) on 8 trn2 cores.

Sharding: core c handles batch b = c//2 and heads (c%2)*8 .. +8.
  - QKV projection column-sharded by head, attention head-sharded,
    c_proj row-sharded; the pair partial sums are combined on host.
Matmuls run in float32r (TF32-like, ~1.3e-4 rel err, 4x fp32 speed).

Self-contained: hardcodes shapes; builds one SPMD Bass program and runs
it on cores 0-7 via run_bass_kernel_spmd.
"""
import math

import numpy as np

import concourse.bass as bass
import concourse.mybir as mybir
import concourse.tile as tile
from concourse.bass_utils import run_bass_kernel_spmd

F32 = mybir.dt.float32
F32R = mybir.dt.float32r
AF = mybir.ActivationFunctionType
ALU = mybir.AluOpType

# problem dims
B, T, C, H = 4, 2048, 2048, 16
HD = 128
NCORES = 8
NH = H // 2          # heads per core
MCH = 512            # matmul moving-operand chunk (max for 4-byte dtypes)

_ctr = [0]


def _legalize_waits(nc, max_waits=1):
    """This walrus build rejects >1 sync wait per instruction. Hoist extra
    waits onto same-engine NoOps inserted directly before the instruction."""
    n_split = 0
    for f in nc.m.functions:
        for blk in f.blocks:
            newil = []
            changed = False
            for inst in blk.instructions:
                si = inst.sync_info
                if si is not None and si.on_wait and len(si.on_wait) > max_waits:
                    waits = list(si.on_wait)
                    for w in waits[:-max_waits]:
                        _ctr[0] += 1
                        nop = mybir.InstNoOp(name=f"I-waitfix-{_ctr[0]}")
                        nop.engine = inst.engine
                        nop.sync_info = mybir.SyncInfo(on_wait=[w], on_update=[])
                        newil.append(nop)
                    inst.sync_info = mybir.SyncInfo(
                        on_wait=waits[-max_waits:], on_update=list(si.on_update)
                    )
                    changed = True
                    n_split += 1
                newil.append(inst)
            if changed:
                blk.instructions = newil
    return n_split


def build_program(T=T, C=C, NH=NH, use_bqkv=False, qtile=512, legalize=True):
    """One core's program: full pipeline for (1 batch, NH heads)."""
    CB = C // 128          # contraction blocks
    TBn = T // 128         # token blocks
    QTILE = min(qtile, T)  # flash tq tile (>=256 for f32r full rate)
    NQT = T // QTILE
    JMAX = QTILE // 128
    DV = NH * 128          # v/proj-shard width
    inv_sqrt_hd = 1.0 / math.sqrt(HD)

    nc = bass.Bass()
    xt_d = nc.dram_tensor("xt", [C, T], F32R, kind="ExternalInput")
    wqk_d = nc.dram_tensor("wqk", [2, NH, 128, C], F32R, kind="ExternalInput")
    wv_d = nc.dram_tensor("wv", [CB, 128, DV], F32R, kind="ExternalInput")
    wp_d = nc.dram_tensor("wp", [NH, 128, C], F32R, kind="ExternalInput")
    cos2_d = nc.dram_tensor("cos2", [128, T], F32, kind="ExternalInput")
    sin2s_d = nc.dram_tensor("sin2s", [128, T], F32, kind="ExternalInput")
    mask_d = nc.dram_tensor("maskbig", [128, 2 * QTILE - 128], F32R, kind="ExternalInput")
    ones_d = nc.dram_tensor("ones128", [128, 128], F32R, kind="ExternalInput")
    if use_bqkv:
        bqk_d = nc.dram_tensor("bqk", [128, 2 * NH], F32, kind="ExternalInput")
        onecol_d = nc.dram_tensor("onecol", [1, 128], F32R, kind="ExternalInput")
        bv_d = nc.dram_tensor("bv", [1, DV], F32R, kind="ExternalInput")
    out_d = nc.dram_tensor("out_partial", [T, C], F32, kind="ExternalOutput")

    q_sp = nc.dram_tensor("q_spill", [NH, 128, T], F32R)
    k_sp = nc.dram_tensor("k_spill", [NH, 128, T], F32R)
    v_sp = nc.dram_tensor("v_spill", [TBn, 128, DV], F32R)

    with tile.TileContext(nc) as tc:
        with (
            tc.tile_pool(name="xpool", bufs=1) as xpool,
            tc.tile_pool(name="cpool", bufs=1) as cpool,
        ):
            xts = []
            for cb in range(CB):
                xt = xpool.tile([128, T], F32R, tag=f"x{cb}")
                nc.sync.dma_start(out=xt[:], in_=xt_d[cb * 128:(cb + 1) * 128, :])
                xts.append(xt)
            if use_bqkv:
                bqk = cpool.tile([128, 2 * NH], F32)
                nc.sync.dma_start(out=bqk[:], in_=bqk_d[:])
                onecol = cpool.tile([1, 128], F32R)
                nc.sync.dma_start(out=onecol[:], in_=onecol_d[:])
                bv = cpool.tile([1, DV], F32R)
                nc.sync.dma_start(out=bv[:], in_=bv_d[:])

            # ---------------- Phase A1: V = x @ Wv (t-major) ----------------
            nhalf = max(1, DV // 512)
            hw = DV // nhalf  # half width (<=512)
            for half in range(nhalf):
                c0 = half * hw
                with (
                    tc.tile_pool(name="wvpool", bufs=1) as wvpool,
                    tc.tile_pool(name="vepool", bufs=2) as vepool,
                    tc.tile_pool(name="psv", bufs=2, space="PSUM") as psvp,
                ):
                    wvts = []
                    for cb in range(CB):
                        wvt = wvpool.tile([128, hw], F32R, tag=f"wv{cb}")
                        nc.sync.dma_start(out=wvt[:], in_=wv_d[cb, :, c0:c0 + hw])
                        wvts.append(wvt)
                    for tb in range(TBn):
                        psv = psvp.tile([128, hw], F32, tag="psv")
                        for cb in range(CB):
                            nc.tensor.matmul(
                                psv[:],
                                xts[cb][:, tb * 128:(tb + 1) * 128],
                                wvts[cb][:],
                                start=(cb == 0),
                                stop=(cb == CB - 1 and not use_bqkv),
                            )
                        if use_bqkv:
                            nc.tensor.matmul(psv[:], onecol[:], bv[:, c0:c0 + hw],
                                             start=False, stop=True)
                        vsb = vepool.tile([128, hw], F32R, tag="vsb")
                        nc.scalar.copy(out=vsb[:], in_=psv[:])
                        nc.sync.dma_start(out=v_sp[tb, :, c0:c0 + hw], in_=vsb[:])

            # ------------- Phase A2: q^T, k^T per head + RoPE -------------
            with (
                tc.tile_pool(name="cspool", bufs=1) as cspool,
                tc.tile_pool(name="qepool", bufs=2) as qepool,
                tc.tile_pool(name="wqpool", bufs=2) as wqpool,
                tc.tile_pool(name="psq", bufs=2, space="PSUM") as psqp,
            ):
                cos2 = cspool.tile([128, T], F32)
                nc.sync.dma_start(out=cos2[:], in_=cos2_d[:])
                sin2s = cspool.tile([128, T], F32)
                nc.sync.dma_start(out=sin2s[:], in_=sin2s_d[:])
                for s in range(2):
                    spill = q_sp if s == 0 else k_sp
                    for h in range(NH):
                        wq = wqpool.tile([128, C], F32R, tag="wq")
                        nc.sync.dma_start(out=wq[:], in_=wqk_d[s, h])
                        ps = psqp.tile([128, T], F32, tag="psq")
                        for t0 in range(0, T, MCH):
                            for cb in range(CB):
                                nc.tensor.matmul(
                                    ps[:, t0:t0 + MCH],
                                    wq[:, cb * 128:(cb + 1) * 128],
                                    xts[cb][:, t0:t0 + MCH],
                                    start=(cb == 0),
                                    stop=(cb == CB - 1),
                                )
                        hw2 = T // 2
                        for half in range(2):
                            hs = slice(half * hw2, (half + 1) * hw2)
                            qb = qepool.tile([128, hw2], F32, tag="qb")
                            if use_bqkv:
                                nc.vector.tensor_scalar(
                                    qb[:], ps[:, hs], bqk[:, s * NH + h:s * NH + h + 1],
                                    None, ALU.add)
                            else:
                                nc.scalar.copy(out=qb[:], in_=ps[:, hs])
                            qrot = qepool.tile([128, hw2], F32, tag="qrot")
                            nc.sync.dma_start(out=qrot[0:64, :], in_=qb[64:128, :])
                            nc.sync.dma_start(out=qrot[64:128, :], in_=qb[0:64, :])
                            nc.vector.tensor_mul(qb[:], qb[:], cos2[:, hs])
                            nc.vector.tensor_mul(qrot[:], qrot[:], sin2s[:, hs])
                            qr = qepool.tile([128, hw2], F32R, tag="qr")
                            nc.vector.tensor_add(qr[:], qb[:], qrot[:])
                            nc.sync.dma_start(out=spill[h, :, hs], in_=qr[:])

        # ---------------- Phase B: causal flash attention ----------------
        # O^T stays resident in SBUF across Phase B -> C (no DRAM round-trip)
        opersist_cm = tc.tile_pool(name="opersist", bufs=1)
        opersist = opersist_cm.__enter__()
        ohs = [opersist.tile([128, T], F32R, name=f"oh{hd}", tag=f"oh{hd}") for hd in range(NH)]
        with (
            tc.tile_pool(name="fpool", bufs=2) as fpool,
            tc.tile_pool(name="bcpool", bufs=1) as bcpool,
            tc.tile_pool(name="ppool", bufs=6) as ppool,
            tc.tile_pool(name="ropool", bufs=2) as ropool,
            tc.tile_pool(name="psS", bufs=4, space="PSUM") as psSp,
            tc.tile_pool(name="psO", bufs=2, space="PSUM") as psOp,
            tc.tile_pool(name="psR", bufs=2, space="PSUM") as psRp,
        ):
            maskt = bcpool.tile([128, 2 * QTILE - 128], F32R)
            nc.sync.dma_start(out=maskt[:], in_=mask_d[:])
            ones = bcpool.tile([128, 128], F32R)
            nc.sync.dma_start(out=ones[:], in_=ones_d[:])
            for h in range(NH):
                qr = fpool.tile([128, T], F32R, tag="qrh")
                nc.sync.dma_start(out=qr[:], in_=q_sp[h])
                kr = fpool.tile([128, T], F32R, tag="krh")
                nc.sync.dma_start(out=kr[:], in_=k_sp[h])
                vh = fpool.tile([128, TBn, 128], F32R, tag="vh")
                nc.sync.dma_start(
                    out=vh[:],
                    in_=v_sp[:, :, h * 128:(h + 1) * 128].transpose([1, 0, 2]))
                for qt in range(NQT):
                    ntk = (qt + 1) * JMAX
                    tqs = slice(qt * QTILE, (qt + 1) * QTILE)
                    psO = psOp.tile([128, QTILE], F32, tag="psO")
                    psR = psRp.tile([128, QTILE], F32, tag="psR")
                    for tkb in range(ntk):
                        psS = psSp.tile([128, QTILE], F32, tag="psS")
                        nc.tensor.matmul(
                            psS[:], kr[:, tkb * 128:(tkb + 1) * 128], qr[:, tqs],
                            start=True, stop=True)
                        pt = ppool.tile([128, QTILE], F32R, tag="pt")
                        nc.scalar.activation(pt[:], psS[:], AF.Exp, scale=inv_sqrt_hd)
                        j = tkb - qt * JMAX
                        if j >= 0:
                            m0 = (JMAX - 1 - j) * 128
                            nc.vector.tensor_mul(pt[:], pt[:], maskt[:, m0:m0 + QTILE])
                        nc.tensor.matmul(psO[:], vh[:, tkb, :], pt[:],
                                         start=(tkb == 0), stop=(tkb == ntk - 1))
                        nc.tensor.matmul(psR[:], ones[:], pt[:],
                                         start=(tkb == 0), stop=(tkb == ntk - 1))
                    rec = ropool.tile([128, QTILE], F32, tag="rec")
                    nc.vector.reciprocal(rec[:], psR[:])
                    nc.vector.tensor_mul(ohs[h][:, tqs], psO[:], rec[:])

        # ---------------- Phase C: out_partial = O @ Wp_shard ----------------
        with (
            tc.tile_pool(name="wppool", bufs=1) as wppool,
            tc.tile_pool(name="oepool", bufs=2) as oepool,
            tc.tile_pool(name="psP", bufs=2, space="PSUM") as psPp,
        ):
            wps = []
            for hd in range(NH):
                wpt = wppool.tile([128, C], F32R, tag=f"wp{hd}")
                nc.sync.dma_start(out=wpt[:], in_=wp_d[hd])
                wps.append(wpt)
            for tb in range(TBn):
                psP = psPp.tile([128, C], F32, tag="psP")
                for c0 in range(0, C, MCH):
                    for hd in range(NH):
                        nc.tensor.matmul(
                            psP[:, c0:c0 + MCH],
                            ohs[hd][:, tb * 128:(tb + 1) * 128],
                            wps[hd][:, c0:c0 + MCH],
                            start=(hd == 0), stop=(hd == NH - 1))
                outsb = oepool.tile([128, C], F32, tag="outsb")
                nc.scalar.copy(out=outsb[:], in_=psP[:])
                nc.sync.dma_start(out=out_d[tb * 128:(tb + 1) * 128, :], in_=outsb[:])
        opersist_cm.__exit__(None, None, None)

    if legalize:
        _legalize_waits(nc)
    return nc


# ---------------------------------------------------------------- host side

_PERM = np.concatenate([np.arange(0, HD, 2), np.arange(1, HD, 2)])  # de-interleave


def shard_core(core, x, freqs_cos, freqs_sin, Wqkv, bqkv, Wproj,
               T=T, C=C, NH=NH, qtile=512, use_bqkv=False):
    """Build the in_map for one core."""
    CB = C // 128
    DV = NH * 128
    QTILE = min(qtile, T)
    b = core // 2
    hb = (core % 2) * NH

    xt = np.ascontiguousarray(x[b].T).astype(np.float32)

    # [2, NH, 128] column indices (q/k, de-interleaved within each head)
    cols = (np.arange(2)[:, None, None] * C
            + (hb + np.arange(NH))[None, :, None] * HD + _PERM[None, None, :])
    wqk = Wqkv[:, cols]                              # [C, 2, NH, 128]
    wqk = np.ascontiguousarray(
        wqk.reshape(CB, 128, 2, NH, 128).transpose(2, 3, 1, 0, 4)
        .reshape(2, NH, 128, C))

    wv = np.ascontiguousarray(
        Wqkv[:, 2 * C + hb * HD: 2 * C + (hb + NH) * HD].reshape(CB, 128, DV))
    wp = np.ascontiguousarray(
        Wproj[hb * HD:(hb + NH) * HD, :].reshape(NH, 128, C))

    cos2 = np.concatenate([freqs_cos.T, freqs_cos.T], 0).astype(np.float32)
    cos2 = np.ascontiguousarray(cos2)                # [128, T]
    sin2s = np.concatenate([-freqs_sin.T, freqs_sin.T], 0).astype(np.float32)
    sin2s = np.ascontiguousarray(sin2s)

    u = np.arange(2 * QTILE - 128)[None, :]
    p = np.arange(128)[:, None]
    maskbig = (p <= u - (QTILE - 128)).astype(np.float32)

    im = {
        "xt": xt, "wqk": wqk, "wv": wv, "wp": wp,
        "cos2": cos2, "sin2s": sin2s, "maskbig": maskbig,
        "ones128": np.ones((128, 128), np.float32),
    }
    if use_bqkv:
        bqk = np.empty((128, 2 * NH), np.float32)
        for s in range(2):
            for h in range(NH):
                bqk[:, s * NH + h] = bqkv[s * C + (hb + h) * HD + _PERM]
        im["bqk"] = bqk
        im["onecol"] = np.ones((1, 128), np.float32)
        im["bv"] = np.ascontiguousarray(
            bqkv[2 * C + hb * HD: 2 * C + (hb + NH) * HD][None, :])
    return im


_CACHE = {}


def _get_program(use_bqkv):
    key = use_bqkv
    if key not in _CACHE:
        _CACHE[key] = build_program(use_bqkv=use_bqkv)
    return _CACHE[key]


def kernel(x, freqs_cos, freqs_sin, Wqkv, bqkv, Wproj, bproj):
    x = np.asarray(x, np.float32)
    freqs_cos = np.asarray(freqs_cos, np.float32)
    freqs_sin = np.asarray(freqs_sin, np.float32)
    Wqkv = np.asarray(Wqkv, np.float32)
    bqkv = np.asarray(bqkv, np.float32)
    Wproj = np.asarray(Wproj, np.float32)
    bproj = np.asarray(bproj, np.float32)

    use_bqkv = bool(np.any(bqkv != 0))
    nc = _get_program(use_bqkv)
    in_maps = [
        shard_core(c, x, freqs_cos, freqs_sin, Wqkv, bqkv, Wproj,
                   use_bqkv=use_bqkv)
        for c in range(NCORES)
    ]
    try:
        res = run_bass_kernel_spmd(nc, in_maps, list(range(NCORES))).results
    except Exception:
        # transient device faults have been observed; retry once
        res = run_bass_kernel_spmd(nc, in_maps, list(range(NCORES))).results

    out = np.empty((B, T, C), np.float32)
    for b in range(B):
        out[b] = res[2 * b]["out_partial"] + res[2 * b + 1]["out_partial"]
    out += bproj[None, None, :]
    return out



# revision 33
# speedup vs baseline: 1.3149x; 1.3149x over previous
"""Causal self-attention (B=4, T=2048, C=2048, H=16, HD=128) on 8 trn2 cores.

Sharding: core c handles batch b = c//2 and heads (c%2)*8 .. +8.
  - QKV projection column-sharded by head, attention head-sharded,
    c_proj row-sharded; the pair partial sums are combined on host.

v2: all matmul operands in fp16 (same PE rate as f32r at N>=256, half
DMA/SBUF, 2x DVE element-wise modes, ~5e-4 quantization like tf32).
Key structure:
  - x streamed in t-chunks so the first matmuls start ~12us in.
  - A2 (q/k projection + RoPE) and flash attention interleaved per head
    (flash[h] emitted after A2[h+1]) so exp/softmax work on the scalar
    and vector engines hides under the tensor engine's projection work.
  - softmax row-sum: per-block accumulation on DVE (fp16 adds), one
    ones-matmul per (head, q-tile) for the cross-partition sum -- the
    per-block ones-matmul of v1 is gone (~62us less PE work).
  - flash inner loop: S-matmuls emitted 2 blocks ahead of the O-matmuls
    so the exp chain latency never stalls the PE; masked (diagonal)
    blocks are processed last within each q-tile.
  - RoPE rotate-half via DVE stream_shuffle (host lays out head dims in
    16-blocked real/imag groups so the rotate is quadrant-local).
  - c_proj streams Wp in 512-column chunks (no big preload stall).

Self-contained: hardcodes shapes; builds one SPMD Bass program and runs
it on cores 0-7 via run_bass_kernel_spmd.
"""
import math
from contextlib import ExitStack

import numpy as np

import concourse.bass as bass
import concourse.mybir as mybir
import concourse.tile as tile
from concourse.bass_utils import run_bass_kernel_spmd

F32 = mybir.dt.float32
F16 = mybir.dt.float16
AF = mybir.ActivationFunctionType
ALU = mybir.AluOpType

# problem dims
B, T, C, H = 4, 2048, 2048, 16
HD = 128
NCORES = 8
NH = H // 2          # heads per core
QTILE = 512

# rotate-half partner: within each 32-partition quadrant, swap the low
# 16 (real slots) with the high 16 (imag slots)
SHUF_MASK = list(range(16, 32)) + list(range(16))

_ctr = [0]


def _legalize_waits(nc, max_waits=1):
    """This walrus build rejects >1 sync wait per instruction. Hoist extra
    waits onto same-engine NoOps inserted directly before the instruction."""
    n_split = 0
    for f in nc.m.functions:
        for blk in f.blocks:
            newil = []
            changed = False
            for inst in blk.instructions:
                si = inst.sync_info
                if si is not None and si.on_wait and len(si.on_wait) > max_waits:
                    waits = list(si.on_wait)
                    for w in waits[:-max_waits]:
                        _ctr[0] += 1
                        nop = mybir.InstNoOp(name=f"I-waitfix-{_ctr[0]}")
                        nop.engine = inst.engine
                        nop.sync_info = mybir.SyncInfo(on_wait=[w], on_update=[])
                        newil.append(nop)
                    inst.sync_info = mybir.SyncInfo(
                        on_wait=waits[-max_waits:], on_update=list(si.on_update)
                    )
                    changed = True
                    n_split += 1
                newil.append(inst)
            if changed:
                blk.instructions = newil
    return n_split


def build_program(T=T, C=C, NH=NH, use_bqkv=False, legalize=True):
    """One core's program: full pipeline for (1 batch, NH heads)."""
    CB = C // 128          # contraction blocks
    TBn = T // 128         # token blocks
    NQT = T // QTILE       # flash q-tiles
    JMAX = QTILE // 128    # key blocks per q-tile diagonal
    NTQ = T // 512         # 512-wide t-chunks (x load / A2 granularity)
    DV = NH * 128          # v/proj-shard width
    inv_sqrt_hd = 1.0 / math.sqrt(HD)

    nc = bass.Bass()
    xt_d = nc.dram_tensor("xt", [C // 128, 128, T], F16, kind="ExternalInput")
    wqk_d = nc.dram_tensor("wqk", [2, NH, 128, C], F16, kind="ExternalInput")
    wv_d = nc.dram_tensor("wv", [CB, 128, DV], F16, kind="ExternalInput")
    wp_d = nc.dram_tensor("wp", [NH, 128, C], F16, kind="ExternalInput")
    cos2_d = nc.dram_tensor("cos2", [128, T], F16, kind="ExternalInput")
    sin2s_d = nc.dram_tensor("sin2s", [128, T], F16, kind="ExternalInput")
    mask_d = nc.dram_tensor("maskbig", [128, 2 * QTILE - 128], F16, kind="ExternalInput")
    ones_d = nc.dram_tensor("ones128", [128, 128], F16, kind="ExternalInput")
    if use_bqkv:
        bqk_d = nc.dram_tensor("bqk", [128, 2 * NH], F32, kind="ExternalInput")
        onecol_d = nc.dram_tensor("onecol", [1, 128], F16, kind="ExternalInput")
        bv_d = nc.dram_tensor("bv", [1, DV], F16, kind="ExternalInput")
    out_d = nc.dram_tensor("out_partial", [T, C], F16, kind="ExternalOutput")

    q_sp = nc.dram_tensor("q_spill", [NH, 128, T], F16)
    k_sp = nc.dram_tensor("k_spill", [NH, 128, T], F16)
    v_sp = nc.dram_tensor("v_spill", [TBn, 128, DV], F16)

    with tile.TileContext(nc) as tc, ExitStack() as es:
        xpool = es.enter_context(tc.tile_pool(name="xpool", bufs=1))
        cpool = es.enter_context(tc.tile_pool(name="cpool", bufs=1))
        opersist = es.enter_context(tc.tile_pool(name="opersist", bufs=1))
        fpool = es.enter_context(tc.tile_pool(name="fpool", bufs=2))
        ppool = es.enter_context(tc.tile_pool(name="ppool", bufs=3))
        rapool = es.enter_context(tc.tile_pool(name="rapool", bufs=1))
        ropool = es.enter_context(tc.tile_pool(name="ropool", bufs=1))

        # x and wv loaded as a few large batched DMAs (HWDGE issue is serial,
        # ~625ns/DMA), first chunks first so A1 compute starts ~7us in
        xa = xpool.tile([128, CB, T], F16, name="xa", tag="xa")
        hw = min(256, DV)
        nq_a1 = DV // hw
        wva = xpool.tile([128, CB, DV], F16, name="wva", tag="wva")
        xc0 = slice(0, 256)
        nc.sync.dma_start(out=xa[:, :, xc0], in_=xt_d[:, :, xc0].transpose([1, 0, 2]))
        for q in range(nq_a1):
            c0 = q * hw
            nc.sync.dma_start(out=wva[:, :, c0:c0 + hw],
                              in_=wv_d[:, :, c0:c0 + hw].transpose([1, 0, 2]))
        for xc in range(1, T // 256):
            ts_ = slice(xc * 256, (xc + 1) * 256)
            nc.sync.dma_start(out=xa[:, :, ts_], in_=xt_d[:, :, ts_].transpose([1, 0, 2]))

        cos2 = cpool.tile([128, T], F16)
        nc.sync.dma_start(out=cos2[:], in_=cos2_d[:])
        sin2s = cpool.tile([128, T], F16)
        nc.sync.dma_start(out=sin2s[:], in_=sin2s_d[:])
        maskt = cpool.tile([128, 2 * QTILE - 128], F16)
        nc.sync.dma_start(out=maskt[:], in_=mask_d[:])
        ones = cpool.tile([128, 128], F16)
        nc.sync.dma_start(out=ones[:], in_=ones_d[:])
        if use_bqkv:
            bqk = cpool.tile([128, 2 * NH], F32)
            nc.sync.dma_start(out=bqk[:], in_=bqk_d[:])
            onecol = cpool.tile([1, 128], F16)
            nc.sync.dma_start(out=onecol[:], in_=onecol_d[:])
            bv = cpool.tile([1, DV], F16)
            nc.sync.dma_start(out=bv[:], in_=bv_d[:])

        a2_es = ExitStack()
        wqpool = a2_es.enter_context(tc.tile_pool(name="wqpool", bufs=3))
        qepool = a2_es.enter_context(tc.tile_pool(name="qepool", bufs=2))

        wq_pre = {}

        def prefetch_wq(h):
            for s in range(2):
                wq = wqpool.tile([128, C], F16, name="wq", tag="wq")
                nc.sync.dma_start(out=wq[:], in_=wqk_d[s, h])
                wq_pre[(s, h)] = wq

        prefetch_wq(0)

        # ---------------- Phase A1: V = x @ Wv (t-major) ----------------
        with (
            tc.tile_pool(name="vepool", bufs=3) as vepool,
            tc.tile_pool(name="psv", bufs=6, space="PSUM") as psvp,
        ):
            for tb in range(TBn):
                vsb = vepool.tile([128, DV], F16, tag="vsb")
                for q in range(nq_a1):
                    c0 = q * hw
                    psv = psvp.tile([128, hw], F32, tag="psv")
                    for cb in range(CB):
                        nc.tensor.matmul(
                            psv[:],
                            xa[:, cb, tb * 128:(tb + 1) * 128],
                            wva[:, cb, c0:c0 + hw],
                            start=(cb == 0),
                            stop=(cb == CB - 1 and not use_bqkv),
                        )
                    if use_bqkv:
                        nc.tensor.matmul(psv[:], onecol[:], bv[:, c0:c0 + hw],
                                         start=False, stop=True)
                    nc.scalar.copy(out=vsb[:, c0:c0 + hw], in_=psv[:])
                nc.sync.dma_start(out=v_sp[tb], in_=vsb[:])

        # --------- Phase A2 (q/k proj + RoPE) + flash, head-interleaved ---------
        ohs = [opersist.tile([128, T], F16, name=f"oh{h}", tag=f"oh{h}")
               for h in range(NH)]
        psSp = a2_es.enter_context(tc.tile_pool(name="psS", bufs=3, space="PSUM"))
        psOp = a2_es.enter_context(tc.tile_pool(name="psO", bufs=2, space="PSUM"))
        psRp = a2_es.enter_context(tc.tile_pool(name="psR", bufs=1, space="PSUM"))
        psqp = a2_es.enter_context(tc.tile_pool(name="psq", bufs=2, space="PSUM"))

        wcpool = a2_es.enter_context(tc.tile_pool(name="wcpool", bufs=2))
        oepool = a2_es.enter_context(tc.tile_pool(name="oepool", bufs=2))

        def gen_a2(h):
            """q/k projection + RoPE for one head; yields after each quarter."""
            if h + 1 < NH:
                prefetch_wq(h + 1)
            if h >= 1:
                prefetch_flash(h - 1)
            for s in range(2):
                spill = q_sp if s == 0 else k_sp
                wq = wq_pre.pop((s, h))
                for tq in range(NTQ):
                    ts_ = slice(tq * 512, (tq + 1) * 512)
                    psq = psqp.tile([128, 512], F32, tag="psq")
                    for cb in range(CB):
                        nc.tensor.matmul(
                            psq[:],
                            wq[:, cb * 128:(cb + 1) * 128],
                            xa[:, cb, ts_],
                            start=(cb == 0),
                            stop=(cb == CB - 1),
                        )
                    qb = qepool.tile([128, 512], F16, tag="qb")
                    if use_bqkv:
                        nc.vector.tensor_scalar(
                            qb[:], psq[:], bqk[:, s * NH + h:s * NH + h + 1],
                            None, ALU.add)
                    else:
                        nc.scalar.copy(out=qb[:], in_=psq[:])
                    qrot = qepool.tile([128, 512], F16, tag="qrot")
                    nc.vector.stream_shuffle(qrot[:], qb[:], SHUF_MASK)
                    nc.vector.tensor_mul(qb[:], qb[:], cos2[:, ts_])
                    nc.vector.tensor_mul(qrot[:], qrot[:], sin2s[:, ts_])
                    qr = qepool.tile([128, 512], F16, tag="qr")
                    nc.vector.tensor_add(qr[:], qb[:], qrot[:])
                    nc.sync.dma_start(out=spill[h, :, ts_], in_=qr[:])
                    yield

        fl_pre = {}

        def prefetch_flash(h):
            # chunked so each piece only waits its own spill quarter
            qr = fpool.tile([128, T], F16, name="qrh", tag="qrh")
            kr = fpool.tile([128, T], F16, name="krh", tag="krh")
            vh = fpool.tile([128, TBn, 128], F16, name="vh", tag="vh")
            nc.sync.dma_start(
                out=vh[:],
                in_=v_sp[:, :, h * 128:(h + 1) * 128].transpose([1, 0, 2]))
            for tq in range(NTQ):
                ts_ = slice(tq * 512, (tq + 1) * 512)
                nc.sync.dma_start(out=qr[:, ts_], in_=q_sp[h][:, ts_])
                nc.sync.dma_start(out=kr[:, ts_], in_=k_sp[h][:, ts_])
            fl_pre[h] = (qr, kr, vh)

        def gen_flash(h):
            """flash attention for one head; yields after each key block.

            Diagonal key blocks only compute the causally-needed column
            range [j*128, QTILE) -- the rest of S/exp/O is skipped.
            """
            qr, kr, vh = fl_pre.pop(h)
            for qt in range(NQT):
                ntk = (qt + 1) * JMAX
                base = qt * JMAX
                tq0 = qt * QTILE

                def col0(i):
                    return max(0, i - base) * 128

                psO = psOp.tile([128, QTILE], F32, tag="psO")
                psS_t = [None] * ntk

                def emit_S(i):
                    c0 = col0(i)
                    psS_t[i] = psSp.tile([128, QTILE], F32, name="psS", tag="psS")
                    nc.tensor.matmul(
                        psS_t[i][:, c0:], kr[:, i * 128:(i + 1) * 128],
                        qr[:, tq0 + c0:tq0 + QTILE], start=True, stop=True)

                for i0 in range(min(3, ntk)):
                    emit_S(i0)
                one_acc = ntk <= JMAX  # qt0: partial blocks from i=1 on
                racc = [None, None]
                for i in range(ntk):
                    c0 = col0(i)
                    pt = ppool.tile([128, QTILE], F16, tag="pt")
                    nc.scalar.activation(pt[:, c0:], psS_t[i][:, c0:], AF.Exp,
                                         scale=inv_sqrt_hd)
                    psS_t[i] = None
                    j = i - base
                    if j >= 0:
                        m0 = (JMAX - 1 - j) * 128
                        nc.vector.tensor_mul(pt[:, c0:], pt[:, c0:],
                                             maskt[:, m0 + c0:m0 + QTILE])
                    a = 0 if one_acc else i % 2
                    if racc[a] is None:
                        assert c0 == 0
                        racc[a] = rapool.tile([128, QTILE], F16, name=f"racc{a}", tag=f"racc{a}")
                        nc.vector.tensor_copy(out=racc[a][:], in_=pt[:])
                    else:
                        nc.vector.tensor_add(racc[a][:, c0:], racc[a][:, c0:],
                                             pt[:, c0:])
                    if i + 3 < ntk:
                        emit_S(i + 3)
                    nc.tensor.matmul(psO[:, c0:], vh[:, i, :], pt[:, c0:],
                                     start=(i == 0), stop=(i == ntk - 1))
                    if i + 1 < ntk:
                        yield
                if racc[1] is not None:
                    nc.vector.tensor_add(racc[0][:], racc[0][:], racc[1][:])
                psR = psRp.tile([128, QTILE], F32, tag="psR")
                nc.tensor.matmul(psR[:], ones[:], racc[0][:], start=True, stop=True)
                rec = ropool.tile([128, QTILE], F32, tag="rec")
                nc.vector.reciprocal(rec[:], psR[:])
                nc.vector.tensor_mul(ohs[h][:, tq0:tq0 + QTILE], psO[:], rec[:])
                yield

        def gen_c():
            """c_proj; yields after each (c-chunk, tb) unit."""
            for c0 in range(0, C, 512):
                wpc = wcpool.tile([128, NH, 512], F16, name="wpc", tag="wpc")
                for hd in range(NH):
                    nc.sync.dma_start(out=wpc[:, hd, :],
                                      in_=wp_d[hd][:, c0:c0 + 512])
                for tb in range(TBn):
                    psP = psqp.tile([128, 512], F32, tag="psq")
                    for hd in range(NH):
                        nc.tensor.matmul(
                            psP[:],
                            ohs[hd][:, tb * 128:(tb + 1) * 128],
                            wpc[:, hd, :],
                            start=(hd == 0), stop=(hd == NH - 1))
                    outsb = oepool.tile([128, 512], F16, tag="outsb")
                    nc.scalar.copy(out=outsb[:], in_=psP[:])
                    nc.sync.dma_start(
                        out=out_d[tb * 128:(tb + 1) * 128, c0:c0 + 512],
                        in_=outsb[:])
                    yield

        def pump(gen, n):
            """Pull up to n units; return False once exhausted."""
            for _ in range(n):
                try:
                    next(gen)
                except StopIteration:
                    return False
            return True

        # A2 head 0 runs alone; flash[h] is woven into A2[h+1] at block
        # granularity so its exp/softmax chain hides under projection matmuls.
        FLASH_UNITS = sum((qt + 1) * JMAX for qt in range(NQT)) + NQT
        A2_UNITS = 2 * NTQ
        for _ in gen_a2(0):
            pass
        for h in range(1, NH):
            ga, gf = gen_a2(h), gen_flash(h - 1)
            per = FLASH_UNITS / A2_UNITS
            acc = 0.0
            for _ in ga:
                acc += per
                take = int(acc)
                acc -= take
                pump(gf, take)
            while pump(gf, 1):
                pass
        # last head's flash woven into c_proj; C's (chunk0, tb) needs
        # ohs[last][tb], i.e. flash q-tile tb*128 // QTILE complete.
        prefetch_flash(NH - 1)
        gf, gc = gen_flash(NH - 1), gen_c()
        c_done = 0
        for qt in range(NQT):
            units_qt = (qt + 1) * JMAX + 1
            c_ok = qt * QTILE // 128  # tb units of chunk0 unlocked so far
            per = (c_ok - c_done) / units_qt
            acc = 0.0
            for _ in range(units_qt):
                pump(gf, 1)
                acc += per
                take = int(acc)
                acc -= take
                pump(gc, take)
                c_done += take
        while pump(gc, 1):
            pass
        a2_es.close()

    if legalize:
        _legalize_waits(nc)
    return nc


# ---------------------------------------------------------------- host side

# partition layout for q/k head dims: 16-blocked real/imag groups so the
# RoPE rotate-half is a within-quadrant stream_shuffle.
# partition p = 32*a + b: b<16 -> real part of pair (16a+b); b>=16 -> imag.
_PAIR = (np.arange(128) // 32) * 16 + (np.arange(128) % 32) % 16
_ISIM = (np.arange(128) % 32) >= 16
_PERM = 2 * _PAIR + _ISIM.astype(np.int64)


def shard_core(core, x, freqs_cos, freqs_sin, Wqkv, bqkv, Wproj,
               T=T, C=C, NH=NH, use_bqkv=False):
    """Build the in_map for one core."""
    CB = C // 128
    DV = NH * 128
    b = core // 2
    hb = (core % 2) * NH

    xt = np.ascontiguousarray(x[b].T).astype(np.float16).reshape(CB, 128, T)

    # [2, NH, 128] column indices (q/k, permuted within each head)
    cols = (np.arange(2)[:, None, None] * C
            + (hb + np.arange(NH))[None, :, None] * HD + _PERM[None, None, :])
    wqk = Wqkv[:, cols]                              # [C, 2, NH, 128]
    wqk = np.ascontiguousarray(
        wqk.reshape(CB, 128, 2, NH, 128).transpose(2, 3, 1, 0, 4)
        .reshape(2, NH, 128, C)).astype(np.float16)

    wv = np.ascontiguousarray(
        Wqkv[:, 2 * C + hb * HD: 2 * C + (hb + NH) * HD].reshape(CB, 128, DV)
    ).astype(np.float16)
    wp = np.ascontiguousarray(
        Wproj[hb * HD:(hb + NH) * HD, :].reshape(NH, 128, C)).astype(np.float16)

    cos2 = np.ascontiguousarray(freqs_cos.T[_PAIR]).astype(np.float16)  # [128, T]
    sign = np.where(_ISIM, 1.0, -1.0).astype(np.float32)
    sin2s = np.ascontiguousarray(freqs_sin.T[_PAIR] * sign[:, None]).astype(np.float16)

    u = np.arange(2 * QTILE - 128)[None, :]
    p = np.arange(128)[:, None]
    maskbig = (p <= u - (QTILE - 128)).astype(np.float16)

    im = {
        "xt": xt, "wqk": wqk, "wv": wv, "wp": wp,
        "cos2": cos2, "sin2s": sin2s, "maskbig": maskbig,
        "ones128": np.ones((128, 128), np.float16),
    }
    if use_bqkv:
        bqk = np.empty((128, 2 * NH), np.float32)
        for s in range(2):
            for h in range(NH):
                bqk[:, s * NH + h] = bqkv[s * C + (hb + h) * HD + _PERM]
        im["bqk"] = bqk
        im["onecol"] = np.ones((1, 128), np.float16)
        im["bv"] = np.ascontiguousarray(
            bqkv[2 * C + hb * HD: 2 * C + (hb + NH) * HD][None, :]).astype(np.float16)
    return im


_CACHE = {}


def _get_program(use_bqkv):
    key = use_bqkv
    if key not in _CACHE:
        _CACHE[key] = build_program(use_bqkv=use_bqkv)
    return _CACHE[key]


def kernel(x, freqs_cos, freqs_sin, Wqkv, bqkv, Wproj, bproj):
    x = np.asarray(x, np.float32)
    freqs_cos = np.asarray(freqs_cos, np.float32)
    freqs_sin = np.asarray(freqs_sin, np.float32)
    Wqkv = np.asarray(Wqkv, np.float32)
    bqkv = np.asarray(bqkv, np.float32)
    Wproj = np.asarray(Wproj, np.float32)
    bproj = np.asarray(bproj, np.float32)

    use_bqkv = bool(np.any(bqkv != 0))
    nc = _get_program(use_bqkv)
    in_maps = [
        shard_core(c, x, freqs_cos, freqs_sin, Wqkv, bqkv, Wproj,
                   use_bqkv=use_bqkv)
        for c in range(NCORES)
    ]
    try:
        res = run_bass_kernel_spmd(nc, in_maps, list(range(NCORES))).results
    except Exception:
        # transient device faults have been observed; retry once
        res = run_bass_kernel_spmd(nc, in_maps, list(range(NCORES))).results

    out = np.empty((B, T, C), np.float32)
    for b in range(B):
        out[b] = (res[2 * b]["out_partial"].astype(np.float32)
                  + res[2 * b + 1]["out_partial"].astype(np.float32))
    out += bproj[None, None, :]
    return out
